# revision 3
# baseline (speedup 1.0000x reference)
"""EDC (Schroeder energy-decay-curve) criterion kernel for Trainium2.

Computes  mean(|edc_db(h) - edc_db(target_h)|)  over [256, 8000] where
edc_db is the truncated, first-sample-normalized energy decay curve in dB.

v2 "PE-tail" design (TimelineSim span ~19.9us vs 24.0us for v1):
  * The ENTIRE tail (t in [8000,32000)) sum-of-squares runs on the PE
    via fp8 DoubleRow Gram matmuls over a host-transposed layout: 94
    accumulating matmuls of [128part(t) x 2ktile x 64rows] tiles ->
    G[64,64] psum whose diagonal holds the per-row tail energies.
    This frees ACT/DVE/Pool entirely for the head pipeline.
  * diag(G) extracted with a masked multiply (mask = I*2^-14, folding
    the global scale) + row-reduce on DVE; a PERM matmul broadcasts the
    64 row totals to the 128-partition A/B head layout (PREV, in psum).
  * Head (t in [0,8000), fp16, descending-t A/B chunk layout): first
    two chunks DMA'd before the tail so DVE pre-computes their
    squares (TensorTensor, 2x fp16) + suffix scans during the tail
    stream; remaining three chunks after.  The ACT Ln stream starts
    as soon as PREV lands; per-chunk Ln bias = PREV + swapped chunk
    totals (SLS = scan's last column; swap/roll ops on Pool, which
    cannot touch psum - only DVE reads G/PREVPS).
  * cab = ln(tot_h)-ln(tot_t) from a tiny dedicated f32 Ln of the last
    chunk's t=0 column, so CAB assembly overlaps the last big Ln.
  * Final |x - cab| waves split ACT (Abs+bias+accum over X2A) / DVE
    (subtract + abs-reduce over X2B); host sums the OUTT columns.

Scheduling notes (TimelineSim-derived): every DMA pays ~0.9us sem
latency before consumers see it; the tile scheduler is greedy per
engine, so the tiny bias-chain ops live on the otherwise-idle Pool
(high_priority) to avoid losing races against big scans on DVE.

fp8 tail + fp16 head + fp16 ln output give rel err ~3.9e-3 vs the f32
reference (tolerance 2e-2); validated on 8 cores end-to-end.
"""

from contextlib import ExitStack

import ml_dtypes
import numpy as np

import concourse.bacc as bacc
import concourse.bass as bass
import concourse.mybir as mybir
import concourse.tile as tile
from concourse.bass_utils import run_bass_kernel_spmd

N_CORES = 8
B = 256
RPC = B // N_CORES      # 32 rows per tensor per core
T = 32000
CAP = 8000
TAILN = T - CAP          # 24000 tail samples per row
KTILE = 256              # t-positions per DoubleRow Gram tile
NGT = (TAILN + KTILE - 1) // KTILE  # 94 gram tiles
TAILPAD = NGT * KTILE    # 24064
TPEW = NGT * 128         # 12032 sbuf cols for transposed tail
C_DB = 10.0 / np.log(10.0)
S_DVE = float(2.0 ** -14)

HEAD_CHUNKS = [1200, 800, 1000, 700, 300]   # A/B cols per chunk, sum 4000
N_PRE = 2                                    # chunks DMA'd before the tail
# X2 psum packing: (p0, xoff). chunks 0-1 -> partitions 0:64, 2-4 -> 64:128
HEAD_X2 = [(0, 0), (0, 1200), (64, 0), (64, 1000), (64, 1700)]
X2W = 2000
TPE_SLAB_COLS = [3456, 3456, 3456, 1664]  # tail DMA slabs (sum 12032)
# wave pieces: (x0, x1, engine) over the packed [128, 2000] X2 space
WAVES = [(0, 1536, "act"), (1536, 2000, "dve")]
OUTW = 8

F32 = mybir.dt.float32
F16 = mybir.dt.float16
F8 = mybir.dt.float8e4
ALU = mybir.AluOpType
ACT_FN = mybir.ActivationFunctionType
PM = mybir.MatmulPerfMode


def _mm_pieces(xoff, w):
    out = []
    a = xoff
    while a < xoff + w:
        b = min((a // 512 + 1) * 512, xoff + w)
        out.append((a, b))
        a = b
    return out


def _emit(ctx: ExitStack, tc: "tile.TileContext", out_ap, tpe_ap, head_ap, w_ap):
    nc = tc.nc
    NCK = len(HEAD_CHUNKS)

    xhpool = ctx.enter_context(tc.tile_pool(name="xh", bufs=NCK))
    hpool = ctx.enter_context(tc.tile_pool(name="h", bufs=3))
    wavep = ctx.enter_context(tc.tile_pool(name="wave", bufs=2))
    small = ctx.enter_context(tc.tile_pool(name="small", bufs=1))
    spool = ctx.enter_context(tc.tile_pool(name="sp", bufs=3))
    ppool = ctx.enter_context(tc.tile_pool(name="ps", bufs=1, space="PSUM"))

    TAILPE = small.tile([128, TPEW], F8)
    WM = small.tile([128, 256], F16)     # [:,0:64] selector, [0:64,64:128] mask, [0:64,128:256] perm
    LNF = small.tile([128, 4000], F16)
    INCL = small.tile([128, 4000], F32)
    DMAT = small.tile([128, 64], F16)
    TOT64 = small.tile([128, 1], F16)
    PREV = small.tile([128, 1], F32)
    SLSS = small.tile([128, NCK], F32)
    SWP = small.tile([128, NCK], F32)
    SWU = small.tile([128, NCK], F32)
    BIASK = small.tile([128, NCK], F32)
    T1 = small.tile([128, 1], F32)
    LT = small.tile([128, 1], F32)
    TL1 = small.tile([128, 1], F32)
    CAB = small.tile([128, 1], F32)
    NCAB = small.tile([128, 1], F32)
    OUTT = small.tile([128, OUTW], F32)
    EPSC = small.tile([128, 1], F32)

    # psum order matters: X2A must start bank-aligned (col 0) so the
    # selector matmul pieces split at 512-col bank boundaries.
    X2A = ppool.tile([128, 1536], F32)
    X2B = ppool.tile([128, 512], F32)
    G = ppool.tile([128, 64], F32)
    PREVPS = ppool.tile([128, 64], F32)

    # --- queue head: weights (Pool/SWDGE), early head chunks + tail (SP) ---
    nc.gpsimd.dma_start(WM[:], w_ap)
    off = 0
    pre_xh = {}
    for k in range(N_PRE):
        w = HEAD_CHUNKS[k]
        xh = xhpool.tile([128, w], F16, tag="xh")
        nc.sync.dma_start(xh[:], head_ap[:, off : off + w])
        pre_xh[k] = xh
        off += w
    off_t = 0
    for slab in TPE_SLAB_COLS:
        nc.sync.dma_start(
            TAILPE[:, off_t : off_t + slab], tpe_ap[:, off_t : off_t + slab]
        )
        off_t += slab
    post_xh = {}
    for k in range(N_PRE, NCK):
        w = HEAD_CHUNKS[k]
        xh = xhpool.tile([128, w], F16, tag="xh")
        nc.sync.dma_start(xh[:], head_ap[:, off : off + w])
        post_xh[k] = xh
        off += w

    # --- ACT: steer table to natural_log set ASAP ---
    nc.vector.memset(EPSC[:], 1.0)
    LJ = small.tile([128, 1], F32)
    nc.scalar.activation(LJ[:], EPSC[:], ACT_FN.Ln, bias=EPSC[:])

    nc.vector.memset(SWP[:], 0.0)
    nc.vector.memset(SWU[:], 0.0)
    nc.vector.memset(OUTT[:], 0.0)

    # --- PE: Gram-accumulate the tail (fp8 DoubleRow, 256-deep per tile) ---
    for k in range(NGT):
        a3 = TAILPE[:, k * 128 : (k + 1) * 128].rearrange(
            "p (two f) -> p two f", two=2
        )
        nc.tensor.matmul(
            G[0:64, 0:64], a3, a3,
            start=(k == 0), stop=(k == NGT - 1),
            perf_mode=PM.DoubleRow,
        )

    # --- per-chunk pipeline pieces -------------------------------------
    def sq_scan_sls(k, xh):
        """DVE: square, scan, chunk-total (SLS) + B->A swap copies."""
        w = HEAD_CHUNKS[k]
        o = sum(HEAD_CHUNKS[:k])
        PSQ = hpool.tile([128, w], F16, tag="psq")
        nc.vector.tensor_tensor(PSQ[:], xh[:], xh[:], op=ALU.mult)
        nc.vector.tensor_tensor_scan(
            INCL[:, o : o + w], PSQ[:], PSQ[:], 0.0, op0=ALU.add, op1=ALU.bypass
        )
        with tc.high_priority():
            nc.gpsimd.tensor_scalar(
                SLSS[:, k : k + 1], INCL[:, o + w - 1 : o + w], S_DVE, None, op0=ALU.mult
            )
            nc.gpsimd.tensor_copy(SWP[0:32, k : k + 1], SLSS[32:64, k : k + 1])
            nc.gpsimd.tensor_copy(SWP[64:96, k : k + 1], SLSS[96:128, k : k + 1])

    def bias_k(k):
        """DVE: Ln bias for chunk k (gates Ln_k)."""
        with tc.high_priority():
            if k == 0:
                nc.vector.tensor_tensor(
                    BIASK[:, 0:1], PREVPS[:, 0:1], SWP[:, 0:1], op=ALU.add
                )
            else:
                nc.gpsimd.tensor_tensor(
                    BIASK[:, k : k + 1], PREV[:], SWP[:, k : k + 1], op=ALU.add
                )

    def roll_k(k):
        """Pool: PREV_{k+1} = PREV_k + SLS_k + SWP_k + SWU_k."""
        if k == NCK - 1:
            return
        with tc.high_priority():
            nc.gpsimd.tensor_copy(SWU[32:64, k : k + 1], SLSS[0:32, k : k + 1])
            nc.gpsimd.tensor_copy(SWU[96:128, k : k + 1], SLSS[64:96, k : k + 1])
            nc.gpsimd.tensor_tensor(T1[:], BIASK[:, k : k + 1], SLSS[:, k : k + 1], op=ALU.add)
            nc.gpsimd.tensor_tensor(PREV[:], T1[:], SWU[:, k : k + 1], op=ALU.add)

    def ln_mm(k):
        """ACT Ln + PE selector matmuls for chunk k."""
        w = HEAD_CHUNKS[k]
        o = sum(HEAD_CHUNKS[:k])
        if k == NCK - 1:
            # tiny f32 Ln of the t=0 column (A-halves = ln(row total)) so
            # CAB assembly overlaps the last big Ln below
            with tc.high_priority():
                nc.scalar.activation(
                    LT[:], INCL[:, o + w - 1 : o + w], ACT_FN.Ln,
                    bias=BIASK[:, k : k + 1], scale=S_DVE,
                )
        nc.scalar.activation(
            LNF[:, o : o + w], INCL[:, o : o + w], ACT_FN.Ln,
            bias=BIASK[:, k : k + 1], scale=S_DVE,
        )
        p0, xoff = HEAD_X2[k]
        for a, b in _mm_pieces(xoff, w):
            dst = (X2A[p0 : p0 + 64, a:b] if b <= 1536
                   else X2B[p0 : p0 + 64, a - 1536 : b - 1536])
            nc.tensor.matmul(
                dst, WM[:, 0:64], LNF[:, o + (a - xoff) : o + (b - xoff)],
                start=True, stop=True,
            )

    # --- pre chunks: squares+scans+SLS while the tail streams ---
    for k in range(N_PRE):
        sq_scan_sls(k, pre_xh[k])

    # --- diag(G)*2^-14 on Pool -> row totals on DVE -> PERM matmul ---
    with tc.high_priority():
        nc.vector.tensor_tensor(DMAT[0:64], G[0:64, 0:64], WM[0:64, 64:128], op=ALU.mult)
        with nc.allow_low_precision(reason="row totals ~2.0 scaled; fp16 rounding ~5e-4 cancels between E[t] and cab"):
            nc.vector.tensor_reduce(TOT64[0:64], DMAT[0:64], axis=mybir.AxisListType.X, op=ALU.add)
        nc.tensor.matmul(PREVPS[:, 0:1], WM[0:64, 128:256], TOT64[0:64], start=True, stop=True)

    # --- chunk streams: DVE bias_k as soon as possible, Ln+mm follow ---
    for k in range(N_PRE):
        bias_k(k)
        roll_k(k)
        ln_mm(k)
    for k in range(N_PRE, NCK):
        sq_scan_sls(k, post_xh[k])
        bias_k(k)
        roll_k(k)
        ln_mm(k)

    # --- cab assembly (DVE), overlapping the last Ln/matmul ---
    nc.vector.tensor_copy(TL1[0:32], LT[64:96])
    nc.vector.tensor_tensor(CAB[0:32], LT[0:32], TL1[0:32], op=ALU.subtract)
    nc.vector.tensor_tensor(NCAB[0:32], TL1[0:32], LT[0:32], op=ALU.subtract)
    for d in (32, 64, 96):
        nc.vector.tensor_copy(CAB[d : d + 32], CAB[0:32])
        nc.vector.tensor_copy(NCAB[d : d + 32], NCAB[0:32])

    # --- waves: sum |x - cab| over X2; pieces by readiness ---
    wi = 0
    for x0, x1, eng in WAVES:
        w = x1 - x0
        src = X2A[:, x0:x1] if x1 <= 1536 else X2B[:, x0 - 1536 : x1 - 1536]
        if eng == "act":
            JA = wavep.tile([128, w], F16, tag=f"wa{wi}")
            nc.scalar.activation(
                JA[:], src, ACT_FN.Abs, bias=NCAB[:], accum_out=OUTT[:, wi : wi + 1]
            )
        else:
            JV = wavep.tile([128, w], F16, tag=f"wv{wi}")
            nc.vector.tensor_scalar(JV[:], src, CAB[:], None, op0=ALU.subtract)
            nc.vector.tensor_reduce(
                OUTT[:, wi : wi + 1], JV[:], axis=mybir.AxisListType.X, op=ALU.add,
                apply_absolute_value=True,
            )
        wi += 1

    nc.sync.dma_start(out_ap, OUTT[:])


def build_bass() -> bass.Bass:
    nc = bacc.Bacc("TRN2", target_bir_lowering=False, debug=False)
    tpe = nc.dram_tensor("tpe", [128, TPEW], F8, kind="ExternalInput").ap()
    head = nc.dram_tensor("head", [128, CAP // 2], F16, kind="ExternalInput").ap()
    w = nc.dram_tensor("w", [128, 256], F16, kind="ExternalInput").ap()
    out = nc.dram_tensor("out", [128, OUTW], F32, kind="ExternalOutput").ap()
    with tile.TileContext(nc) as tc, ExitStack() as ctx:
        _emit(ctx, tc, out, tpe, head, w)
    nc.compile()
    return nc


def _w_matrix() -> np.ndarray:
    wm = np.zeros((128, 256), np.float16)
    # selector: out[r] = lnh[r] - lnt[r]
    wm[np.arange(64), np.arange(64)] = 1.0
    wm[64 + np.arange(64), np.arange(64)] = -1.0
    # diag mask I * 2^-14
    wm[np.arange(64), 64 + np.arange(64)] = S_DVE
    # perm: PREV[p] = TOT64[rowmap(p)]; stationary [64, 128] at cols 128:256
    rowmap = np.concatenate([np.arange(32), np.arange(32), 32 + np.arange(32), 32 + np.arange(32)])
    wm[rowmap, 128 + np.arange(128)] = 1.0
    return wm


def _host_layout(hc: np.ndarray, tc_: np.ndarray):
    """32 h + 32 t rows [32, 32000] f32 -> (tpe fp8 [128, 12032] transposed
    Gram layout, head fp16 [128, 4000] descending-t A/B chunks)."""
    rows = np.concatenate([hc, tc_], axis=0)  # [64, 32000]
    tail = np.zeros((64, TAILPAD), dtype=np.float32)
    tail[:, :TAILN] = rows[:, CAP:]
    tail8 = tail.astype(ml_dtypes.float8_e4m3fn)
    # tpe[p, 128k + 64j + r] = tail8[r, 256k + 128j + p]
    t4 = tail8.reshape(64, NGT, 2, 128)           # r, k, j, p
    tpe = np.ascontiguousarray(t4.transpose(3, 1, 2, 0)).reshape(128, TPEW)

    head = np.empty((128, CAP // 2), dtype=np.float16)
    for ti, rr in ((0, hc), (1, tc_)):
        p = 64 * ti
        hi = CAP
        off = 0
        for w in HEAD_CHUNKS:
            lo = hi - 2 * w
            mid = lo + w
            head[p : p + 32, off : off + w] = rr[:, lo:mid][:, ::-1]
            head[p + 32 : p + 64, off : off + w] = rr[:, mid:hi][:, ::-1]
            hi = lo
            off += w
    return tpe, head


_NC_CACHE: list = []


def kernel(h: np.ndarray, target_h: np.ndarray) -> np.ndarray:
    h = np.ascontiguousarray(np.asarray(h, dtype=np.float32).reshape(B, T))
    t = np.ascontiguousarray(np.asarray(target_h, dtype=np.float32).reshape(B, T))

    if not _NC_CACHE:
        _NC_CACHE.append(build_bass())
    nc = _NC_CACHE[0]

    wmat = _w_matrix()
    in_maps = []
    for c in range(N_CORES):
        rows = slice(c * RPC, (c + 1) * RPC)
        tpe, head = _host_layout(h[rows], t[rows])
        in_maps.append({"tpe": tpe, "head": head, "w": wmat})

    res = run_bass_kernel_spmd(nc, in_maps, core_ids=list(range(N_CORES)))
    total = 0.0
    for r in res.results:
        o = r["out"].astype(np.float64)
        total += o[:, 0 : len(WAVES)].sum()
    return np.float32(C_DB * total / (B * CAP))


# revision 4
# speedup vs baseline: 1.0149x; 1.0149x over previous
"""EDC (Schroeder energy-decay-curve) criterion kernel for Trainium2.

Computes  mean(|edc_db(h) - edc_db(target_h)|)  over [256, 8000] where
edc_db is the truncated, first-sample-normalized energy decay curve in dB.

v2 "PE-tail" design (TimelineSim span ~19.9us vs 24.0us for v1):
  * The ENTIRE tail (t in [8000,32000)) sum-of-squares runs on the PE
    via fp8 DoubleRow Gram matmuls over a host-transposed layout: 94
    accumulating matmuls of [128part(t) x 2ktile x 64rows] tiles ->
    G[64,64] psum whose diagonal holds the per-row tail energies.
    This frees ACT/DVE/Pool entirely for the head pipeline.
  * diag(G) extracted with a masked multiply (mask = I*2^-14, folding
    the global scale) + row-reduce on DVE; a PERM matmul broadcasts the
    64 row totals to the 128-partition A/B head layout (PREV, in psum).
  * Head (t in [0,8000), fp16, descending-t A/B chunk layout): first
    two chunks DMA'd before the tail so DVE pre-computes their
    squares (TensorTensor, 2x fp16) + suffix scans during the tail
    stream; remaining three chunks after.  The ACT Ln stream starts
    as soon as PREV lands; per-chunk Ln bias = PREV + swapped chunk
    totals (SLS = scan's last column; swap/roll ops on Pool, which
    cannot touch psum - only DVE reads G/PREVPS).
  * cab = ln(tot_h)-ln(tot_t) from a tiny dedicated f32 Ln of the last
    chunk's t=0 column, so CAB assembly overlaps the last big Ln.
  * Final |x - cab| waves split ACT (Abs+bias+accum over X2A) / DVE
    (subtract + abs-reduce over X2B); host sums the OUTT columns.

Scheduling notes (TimelineSim-derived): every DMA pays ~0.9us sem
latency before consumers see it; the tile scheduler is greedy per
engine, so the tiny bias-chain ops live on the otherwise-idle Pool
(high_priority) to avoid losing races against big scans on DVE.

fp8 tail + fp16 head + fp16 ln output give rel err ~3.9e-3 vs the f32
reference (tolerance 2e-2); validated on 8 cores end-to-end.
"""

from contextlib import ExitStack

import ml_dtypes
import numpy as np

import concourse.bacc as bacc
import concourse.bass as bass
import concourse.mybir as mybir
import concourse.tile as tile
from concourse.bass_utils import run_bass_kernel_spmd

N_CORES = 8
B = 256
RPC = B // N_CORES      # 32 rows per tensor per core
T = 32000
CAP = 8000
TAILN = T - CAP          # 24000 tail samples per row
KTILE = 256              # t-positions per DoubleRow Gram tile
NGT = (TAILN + KTILE - 1) // KTILE  # 94 gram tiles
TAILPAD = NGT * KTILE    # 24064
TPEW = NGT * 128         # 12032 sbuf cols for transposed tail
C_DB = 10.0 / np.log(10.0)
S_DVE = float(2.0 ** -14)

HEAD_CHUNKS = [1200, 800, 1000, 700, 300]   # A/B cols per chunk, sum 4000
N_PRE = 2                                    # chunks DMA'd before the tail
# X2 psum packing: (p0, xoff). chunks 0-1 -> partitions 0:64, 2-4 -> 64:128
HEAD_X2 = [(0, 0), (0, 1200), (64, 0), (64, 1000), (64, 1700)]
X2W = 2000
TPE_SLAB_COLS = [3456, 3456, 3456, 1664]  # tail DMA slabs (sum 12032)
# wave pieces: (x0, x1, engine) over the packed [128, 2000] X2 space
WAVES = [(0, 1536, "act"), (1536, 2000, "dve")]
OUTW = 8

F32 = mybir.dt.float32
F16 = mybir.dt.float16
F8 = mybir.dt.float8e4
ALU = mybir.AluOpType
ACT_FN = mybir.ActivationFunctionType
PM = mybir.MatmulPerfMode


def _mm_pieces(xoff, w):
    out = []
    a = xoff
    while a < xoff + w:
        b = min((a // 512 + 1) * 512, xoff + w)
        out.append((a, b))
        a = b
    return out


def _emit(ctx: ExitStack, tc: "tile.TileContext", out_ap, tpe_ap, head_ap, w_ap):
    nc = tc.nc
    NCK = len(HEAD_CHUNKS)

    xhpool = ctx.enter_context(tc.tile_pool(name="xh", bufs=NCK))
    hpool = ctx.enter_context(tc.tile_pool(name="h", bufs=3))
    wavep = ctx.enter_context(tc.tile_pool(name="wave", bufs=2))
    small = ctx.enter_context(tc.tile_pool(name="small", bufs=1))
    spool = ctx.enter_context(tc.tile_pool(name="sp", bufs=3))
    ppool = ctx.enter_context(tc.tile_pool(name="ps", bufs=1, space="PSUM"))

    TAILPE = small.tile([128, TPEW], F8)
    WM = small.tile([128, 256], F16)     # [:,0:64] selector, [0:64,64:128] mask, [0:64,128:256] perm
    LNF = small.tile([128, 4000], F16)
    INCL = small.tile([128, 4000], F32)
    DMAT = small.tile([128, 64], F16)
    TOT64 = small.tile([128, 1], F16)
    PREV = small.tile([128, 1], F32)
    SLSS = small.tile([128, NCK], F32)
    SWP = small.tile([128, NCK], F32)
    SWU = small.tile([128, NCK], F32)
    BIASK = small.tile([128, NCK], F32)
    T1 = small.tile([128, 1], F32)
    LT = small.tile([128, 1], F32)
    TL1 = small.tile([128, 1], F32)
    CAB = small.tile([128, 1], F32)
    NCAB = small.tile([128, 1], F32)
    OUTT = small.tile([128, OUTW], F32)
    EPSC = small.tile([128, 1], F32)

    # psum order matters: X2A must start bank-aligned (col 0) so the
    # selector matmul pieces split at 512-col bank boundaries.
    X2A = ppool.tile([128, 1536], F32)
    X2B = ppool.tile([128, 512], F32)
    G = ppool.tile([128, 64], F32)
    PREVPS = ppool.tile([128, 64], F32)

    # --- queue head: weights (Pool/SWDGE), early head chunks + tail (SP) ---
    nc.gpsimd.dma_start(WM[:], w_ap)
    off = 0
    pre_xh = {}
    for k in range(N_PRE):
        w = HEAD_CHUNKS[k]
        xh = xhpool.tile([128, w], F16, tag="xh")
        nc.sync.dma_start(xh[:], head_ap[:, off : off + w])
        pre_xh[k] = xh
        off += w
    off_t = 0
    for slab in TPE_SLAB_COLS:
        nc.sync.dma_start(
            TAILPE[:, off_t : off_t + slab], tpe_ap[:, off_t : off_t + slab]
        )
        off_t += slab
    post_xh = {}
    for k in range(N_PRE, NCK):
        w = HEAD_CHUNKS[k]
        xh = xhpool.tile([128, w], F16, tag="xh")
        nc.sync.dma_start(xh[:], head_ap[:, off : off + w])
        post_xh[k] = xh
        off += w

    # --- ACT: steer table to natural_log set ASAP ---
    nc.vector.memset(EPSC[:], 1.0)
    LJ = small.tile([128, 1], F32)
    nc.scalar.activation(LJ[:], EPSC[:], ACT_FN.Ln, bias=EPSC[:])

    nc.vector.memset(SWP[:], 0.0)
    nc.vector.memset(SWU[:], 0.0)
    nc.vector.memset(OUTT[:], 0.0)

    # --- PE: Gram-accumulate the tail (fp8 DoubleRow, 256-deep per tile) ---
    for k in range(NGT):
        a3 = TAILPE[:, k * 128 : (k + 1) * 128].rearrange(
            "p (two f) -> p two f", two=2
        )
        nc.tensor.matmul(
            G[0:64, 0:64], a3, a3,
            start=(k == 0), stop=(k == NGT - 1),
            perf_mode=PM.DoubleRow,
        )

    # --- per-chunk pipeline pieces -------------------------------------
    def sq_scan_sls(k, xh):
        """DVE: square, scan, swapped chunk totals (Pool)."""
        w = HEAD_CHUNKS[k]
        o = sum(HEAD_CHUNKS[:k])
        PSQ = hpool.tile([128, w], F16, tag="psq")
        nc.vector.tensor_tensor(PSQ[:], xh[:], xh[:], op=ALU.mult)
        nc.vector.tensor_tensor_scan(
            INCL[:, o : o + w], PSQ[:], PSQ[:], 0.0, op0=ALU.add, op1=ALU.bypass
        )
        with tc.high_priority():
            # B-half totals straight from the scan's last column with the
            # 2^-14 scale fused: 2 ops gate BIAS_k instead of 3 (SLS is
            # deferred into roll_k, off the scan->Ln critical chain)
            nc.gpsimd.tensor_scalar(
                SWP[0:32, k : k + 1], INCL[32:64, o + w - 1 : o + w], S_DVE, None, op0=ALU.mult
            )
            nc.gpsimd.tensor_scalar(
                SWP[64:96, k : k + 1], INCL[96:128, o + w - 1 : o + w], S_DVE, None, op0=ALU.mult
            )

    def bias_k(k):
        """Ln bias for chunk k (gates Ln_k). k=0 runs on ACT itself
        (Identity reading PREVPS from psum + SWP0 as the bias operand):
        same engine as Ln0, so no cross-engine hop and no DVE greedy race
        against the post-chunk squares."""
        with tc.high_priority():
            if k == 0:
                nc.scalar.activation(
                    BIASK[:, 0:1], PREVPS[:, 0:1], ACT_FN.Identity, bias=SWP[:, 0:1]
                )
            else:
                nc.gpsimd.tensor_tensor(
                    BIASK[:, k : k + 1], PREV[:], SWP[:, k : k + 1], op=ALU.add
                )

    def roll_k(k):
        """Pool: PREV_{k+1} = PREV_k + SLS_k + SWP_k + SWU_k."""
        if k == NCK - 1:
            return
        with tc.high_priority():
            nc.gpsimd.tensor_scalar(
                SLSS[:, k : k + 1], INCL[:, sum(HEAD_CHUNKS[: k + 1]) - 1 : sum(HEAD_CHUNKS[: k + 1])],
                S_DVE, None, op0=ALU.mult
            )
            nc.gpsimd.tensor_copy(SWU[32:64, k : k + 1], SLSS[0:32, k : k + 1])
            nc.gpsimd.tensor_copy(SWU[96:128, k : k + 1], SLSS[64:96, k : k + 1])
            nc.gpsimd.tensor_tensor(T1[:], BIASK[:, k : k + 1], SLSS[:, k : k + 1], op=ALU.add)
            nc.gpsimd.tensor_tensor(PREV[:], T1[:], SWU[:, k : k + 1], op=ALU.add)

    def ln_mm(k):
        """ACT Ln + PE selector matmuls for chunk k."""
        w = HEAD_CHUNKS[k]
        o = sum(HEAD_CHUNKS[:k])
        if k == NCK - 1:
            # tiny f32 Ln of the t=0 column (A-halves = ln(row total)) so
            # CAB assembly overlaps the last big Ln below
            with tc.high_priority():
                nc.scalar.activation(
                    LT[:], INCL[:, o + w - 1 : o + w], ACT_FN.Ln,
                    bias=BIASK[:, k : k + 1], scale=S_DVE,
                )
        nc.scalar.activation(
            LNF[:, o : o + w], INCL[:, o : o + w], ACT_FN.Ln,
            bias=BIASK[:, k : k + 1], scale=S_DVE,
        )
        p0, xoff = HEAD_X2[k]
        for a, b in _mm_pieces(xoff, w):
            dst = (X2A[p0 : p0 + 64, a:b] if b <= 1536
                   else X2B[p0 : p0 + 64, a - 1536 : b - 1536])
            nc.tensor.matmul(
                dst, WM[:, 0:64], LNF[:, o + (a - xoff) : o + (b - xoff)],
                start=True, stop=True,
            )

    # --- pre chunks: squares+scans+SLS while the tail streams ---
    for k in range(N_PRE):
        sq_scan_sls(k, pre_xh[k])

    # --- diag(G)*2^-14 on Pool -> row totals on DVE -> PERM matmul ---
    with tc.high_priority():
        nc.vector.tensor_tensor(DMAT[0:64], G[0:64, 0:64], WM[0:64, 64:128], op=ALU.mult)
        with nc.allow_low_precision(reason="row totals ~2.0 scaled; fp16 rounding ~5e-4 cancels between E[t] and cab"):
            nc.vector.tensor_reduce(TOT64[0:64], DMAT[0:64], axis=mybir.AxisListType.X, op=ALU.add)
        nc.tensor.matmul(PREVPS[:, 0:1], WM[0:64, 128:256], TOT64[0:64], start=True, stop=True)

    # --- chunk streams: DVE bias_k as soon as possible, Ln+mm follow ---
    for k in range(N_PRE):
        bias_k(k)
        roll_k(k)
        ln_mm(k)
    for k in range(N_PRE, NCK):
        sq_scan_sls(k, post_xh[k])
        bias_k(k)
        roll_k(k)
        ln_mm(k)

    # --- cab assembly (DVE), overlapping the last Ln/matmul ---
    nc.vector.tensor_copy(TL1[0:32], LT[64:96])
    nc.vector.tensor_tensor(CAB[0:32], LT[0:32], TL1[0:32], op=ALU.subtract)
    nc.vector.tensor_tensor(NCAB[0:32], TL1[0:32], LT[0:32], op=ALU.subtract)
    for d in (32, 64, 96):
        nc.vector.tensor_copy(CAB[d : d + 32], CAB[0:32])
        nc.vector.tensor_copy(NCAB[d : d + 32], NCAB[0:32])

    # --- waves: sum |x - cab| over X2; pieces by readiness ---
    wi = 0
    for x0, x1, eng in WAVES:
        w = x1 - x0
        src = X2A[:, x0:x1] if x1 <= 1536 else X2B[:, x0 - 1536 : x1 - 1536]
        if eng == "act":
            JA = wavep.tile([128, w], F16, tag=f"wa{wi}")
            nc.scalar.activation(
                JA[:], src, ACT_FN.Abs, bias=NCAB[:], accum_out=OUTT[:, wi : wi + 1]
            )
        else:
            JV = wavep.tile([128, w], F16, tag=f"wv{wi}")
            nc.vector.tensor_scalar(JV[:], src, CAB[:], None, op0=ALU.subtract)
            nc.vector.tensor_reduce(
                OUTT[:, wi : wi + 1], JV[:], axis=mybir.AxisListType.X, op=ALU.add,
                apply_absolute_value=True,
            )
        wi += 1

    nc.sync.dma_start(out_ap, OUTT[:])


def build_bass() -> bass.Bass:
    nc = bacc.Bacc("TRN2", target_bir_lowering=False, debug=False)
    tpe = nc.dram_tensor("tpe", [128, TPEW], F8, kind="ExternalInput").ap()
    head = nc.dram_tensor("head", [128, CAP // 2], F16, kind="ExternalInput").ap()
    w = nc.dram_tensor("w", [128, 256], F16, kind="ExternalInput").ap()
    out = nc.dram_tensor("out", [128, OUTW], F32, kind="ExternalOutput").ap()
    with tile.TileContext(nc) as tc, ExitStack() as ctx:
        _emit(ctx, tc, out, tpe, head, w)
    nc.compile()
    return nc


def _w_matrix() -> np.ndarray:
    wm = np.zeros((128, 256), np.float16)
    # selector: out[r] = lnh[r] - lnt[r]
    wm[np.arange(64), np.arange(64)] = 1.0
    wm[64 + np.arange(64), np.arange(64)] = -1.0
    # diag mask I * 2^-14
    wm[np.arange(64), 64 + np.arange(64)] = S_DVE
    # perm: PREV[p] = TOT64[rowmap(p)]; stationary [64, 128] at cols 128:256
    rowmap = np.concatenate([np.arange(32), np.arange(32), 32 + np.arange(32), 32 + np.arange(32)])
    wm[rowmap, 128 + np.arange(128)] = 1.0
    return wm


def _host_layout(hc: np.ndarray, tc_: np.ndarray):
    """32 h + 32 t rows [32, 32000] f32 -> (tpe fp8 [128, 12032] transposed
    Gram layout, head fp16 [128, 4000] descending-t A/B chunks)."""
    rows = np.concatenate([hc, tc_], axis=0)  # [64, 32000]
    tail = np.zeros((64, TAILPAD), dtype=np.float32)
    tail[:, :TAILN] = rows[:, CAP:]
    tail8 = tail.astype(ml_dtypes.float8_e4m3fn)
    # tpe[p, 128k + 64j + r] = tail8[r, 256k + 128j + p]
    t4 = tail8.reshape(64, NGT, 2, 128)           # r, k, j, p
    tpe = np.ascontiguousarray(t4.transpose(3, 1, 2, 0)).reshape(128, TPEW)

    head = np.empty((128, CAP // 2), dtype=np.float16)
    for ti, rr in ((0, hc), (1, tc_)):
        p = 64 * ti
        hi = CAP
        off = 0
        for w in HEAD_CHUNKS:
            lo = hi - 2 * w
            mid = lo + w
            head[p : p + 32, off : off + w] = rr[:, lo:mid][:, ::-1]
            head[p + 32 : p + 64, off : off + w] = rr[:, mid:hi][:, ::-1]
            hi = lo
            off += w
    return tpe, head


_NC_CACHE: list = []


def kernel(h: np.ndarray, target_h: np.ndarray) -> np.ndarray:
    h = np.ascontiguousarray(np.asarray(h, dtype=np.float32).reshape(B, T))
    t = np.ascontiguousarray(np.asarray(target_h, dtype=np.float32).reshape(B, T))

    if not _NC_CACHE:
        _NC_CACHE.append(build_bass())
    nc = _NC_CACHE[0]

    wmat = _w_matrix()
    in_maps = []
    for c in range(N_CORES):
        rows = slice(c * RPC, (c + 1) * RPC)
        tpe, head = _host_layout(h[rows], t[rows])
        in_maps.append({"tpe": tpe, "head": head, "w": wmat})

    res = run_bass_kernel_spmd(nc, in_maps, core_ids=list(range(N_CORES)))
    total = 0.0
    for r in res.results:
        o = r["out"].astype(np.float64)
        total += o[:, 0 : len(WAVES)].sum()
    return np.float32(C_DB * total / (B * CAP))
